# revision 2
# baseline (speedup 1.0000x reference)
"""Trainium2 Bass kernel for the GRU seq2seq AR model.

Model (reference): GRU encoder over S=1024 steps, then T=256 autoregressive
decoder steps (teacher_forcing_rate=0, so decoder input is always its own
previous output y = Wl @ h + bl).

Strategy:
  - Pure data parallel: batch 512 sharded 8 ways (64 rows/core), weights
    replicated, zero collectives.
  - Per step, fused matmul groups compute all gate pre-activations with
    M-columns laid out as [r|z] (fp8 e4m3 weights + fp8 activations; the
    sigmoid path tolerates e4m3 — measured rel err 0.0044 vs 0.0026 all-bf16)
    and [i_n|h_n|y] (bf16). A ones-row folds all biases into the matmul.
    r/z/h_n weight columns are pre-scaled by 0.5 so gates reduce to tanh only
    (sigmoid(a) = .5 + .5*tanh(.5a)), avoiding ACT table switches.
  - fp8 r/z tiles halve LDWEIGHTS time (FWL 4 elem/cycle vs 2 for bf16),
    which is the PE bottleneck at N=64 moving columns.
  - Decoder: Wl is FOLDED into the gate weights (inp = Wl@h + bl always), so
    the decoder has no serial y-feedback; the y chunk is output-only.
  - Gate math: h' = h + (1-z)*(tanh(an) - h), 6 DVE ops + 2-half tanh.
  - fp32 PSUM + fp32/bf16 gate math + bf16 hidden master + fp8 shadow.

Layouts (per core, BC = 64):
  w_n  DRAM [10, 128, 1088] bf16: cols [i_n 512 | h_n 512 | y 64];
       k-tiles 0:5 encoder, 5:10 decoder.
  w_rz DRAM [10, 128, 1024] fp8e4: cols [r 512 | z 512], 0.5-scaled.
  xh   DRAM [65, S*BC] bf16: rows 0:64 = x[t].T steps, row 64 = ones.
  xh8  DRAM [65, S*BC] fp8e4: same, for the r/z matmuls.
  y    DRAM [64, (T+1)*BC] f32: slot d holds Wl@h^{(d)}+bl ([I, BC] each).
"""

import numpy as np
import ml_dtypes

B, S, I, H, T = 512, 1024, 64, 512, 256
NCORES = 8
BC = B // NCORES
BF16 = ml_dtypes.bfloat16
FP8 = ml_dtypes.float8_e4m3

# column offsets
_N0, _H0, _Y0 = 0, 512, 1024      # within w_n tiles (1088 cols)
_R0, _Z0 = 0, 512                 # within w_rz tiles (1024 cols)
WN_COLS, WRZ_COLS = 1088, 1024


def _build_weights(Wi, Wh, bi, bh, Wl, bl):
    """K rows: k0 = [x(64); ones(1)], k1..k4 = h chunks of 128.
    Tiles 0:5 = encoder, 5:10 = decoder (Wl folded)."""
    wn = np.zeros((10, 128, WN_COLS), np.float32)
    wrz = np.zeros((10, 128, WRZ_COLS), np.float32)

    def fill(base, Wx, bx_r, bx_z, bx_n, Whh, Win_, x_has_w):
        if x_has_w:
            wrz[base, 0:64, _R0:_R0 + 512] = 0.5 * Wx.T[:, 0:512]
            wrz[base, 0:64, _Z0:_Z0 + 512] = 0.5 * Wx.T[:, 512:1024]
            wn[base, 0:64, _N0:_N0 + 512] = Wx.T[:, 1024:1536]
        wrz[base, 64, _R0:_R0 + 512] = 0.5 * bx_r
        wrz[base, 64, _Z0:_Z0 + 512] = 0.5 * bx_z
        wn[base, 64, _N0:_N0 + 512] = bx_n
        wn[base, 64, _H0:_H0 + 512] = 0.5 * bh[1024:1536]
        wn[base, 64, _Y0:_Y0 + 64] = bl
        for c in range(4):
            hs = slice(128 * c, 128 * (c + 1))
            wrz[base + 1 + c, :, _R0:_R0 + 512] = 0.5 * Whh.T[hs, 0:512]
            wrz[base + 1 + c, :, _Z0:_Z0 + 512] = 0.5 * Whh.T[hs, 512:1024]
            if Win_ is not None:
                wn[base + 1 + c, :, _N0:_N0 + 512] = Win_.T[hs, :]
            wn[base + 1 + c, :, _H0:_H0 + 512] = 0.5 * Wh[1024:1536].T[hs, :]
            wn[base + 1 + c, :, _Y0:_Y0 + 64] = Wl.T[hs, :]

    # encoder: gi from x via k0; gh from h
    fill(0, Wi, (bi + bh)[0:512], (bi + bh)[512:1024], bi[1024:1536],
         Wh[0:1024], None, x_has_w=True)
    # decoder: inp = Wl@h + bl folded -> all through h rows
    Wc = Wh[0:1024] + Wi[0:1024] @ Wl
    Win = Wi[1024:1536] @ Wl
    fill(5, Wi,
         (bi + bh)[0:512] + Wi[0:512] @ bl,
         (bi + bh)[512:1024] + Wi[512:1024] @ bl,
         bi[1024:1536] + Wi[1024:1536] @ bl,
         Wc, Win, x_has_w=False)
    return wn.astype(BF16), wrz.astype(FP8)


def _build_x(xc):
    """xc [BC, S, I] -> ([65, S*BC] bf16, [65, S*BC] fp8) with ones row."""
    s = xc.shape[1]
    xt = np.ones((65, s, BC), np.float32)
    xt[0:64] = xc.transpose(2, 1, 0)
    flat = np.ascontiguousarray(xt.reshape(65, s * BC))
    return flat.astype(BF16), flat.astype(FP8)


def build_program(s_steps=S, t_steps=T, ue=256, ud=64, use_loops=True):
    """Build the Bass/Tile program (shared by all 8 cores)."""
    from contextlib import ExitStack
    import concourse.bass as bass
    import concourse.bacc as bacc
    import concourse.mybir as mybir
    import concourse.tile as tile

    f32 = mybir.dt.float32
    bf16 = mybir.dt.bfloat16
    fp8 = mybir.dt.float8e4
    TANH = mybir.ActivationFunctionType.Tanh
    MUL = mybir.AluOpType.mult
    ADD = mybir.AluOpType.add
    SUB = mybir.AluOpType.subtract

    assert s_steps % ue == 0 and t_steps % ud == 0

    nc = bacc.Bacc("TRN2", target_bir_lowering=False, debug=False,
                   num_devices=NCORES)
    wn_ext = nc.declare_dram_parameter("w_n", [10, 128, WN_COLS], bf16,
                                       isOutput=False)
    wrz_ext = nc.declare_dram_parameter("w_rz", [10, 128, WRZ_COLS], fp8,
                                        isOutput=False)
    x_ext = nc.declare_dram_parameter("xh", [65, s_steps * BC], bf16,
                                      isOutput=False)
    x8_ext = nc.declare_dram_parameter("xh8", [65, s_steps * BC], fp8,
                                       isOutput=False)
    y_ext = nc.declare_dram_parameter("y", [64, (t_steps + 1) * BC], f32,
                                      isOutput=True)

    with ExitStack() as ctx:
        tc = ctx.enter_context(tile.TileContext(nc))
        state = ctx.enter_context(tc.tile_pool(name="state", bufs=1))
        wpool = ctx.enter_context(tc.tile_pool(name="wpool", bufs=1))
        xpool = ctx.enter_context(tc.tile_pool(name="xpool", bufs=2))
        ypool = ctx.enter_context(tc.tile_pool(name="ypool", bufs=2))
        gp = ctx.enter_context(tc.tile_pool(name="gates", bufs=3))
        psum = ctx.enter_context(tc.tile_pool(name="psum", bufs=2, space="PSUM"))

        wne, wnd, wrze, wrzd = [], [], [], []
        for k in range(10):
            tn_ = wpool.tile([128, WN_COLS], bf16, tag=f"wn{k}")
            nc.sync.dma_start(tn_[:], wn_ext[k, :, :])
            trz_ = wpool.tile([128, WRZ_COLS], fp8, tag=f"wrz{k}")
            nc.sync.dma_start(trz_[:], wrz_ext[k, :, :])
            (wne if k < 5 else wnd).append(tn_)
            (wrze if k < 5 else wrzd).append(trz_)

        hbf = state.tile([128, 256], bf16, tag="hbf")     # h.T chunks (bf16)
        hbf8 = state.tile([128, 256], fp8, tag="hbf8")    # fp8 shadow
        rhs0d = state.tile([65, BC], bf16, tag="rhs0d")   # dec k0 = [0...; 1]
        rhs0d8 = state.tile([65, BC], fp8, tag="rhs0d8")
        nc.vector.memset(hbf[:], 0.0)
        nc.vector.memset(hbf8[:], 0.0)
        nc.vector.memset(rhs0d[:], 0.0)
        nc.vector.memset(rhs0d[64:65, :], 1.0)
        nc.vector.memset(rhs0d8[:], 0.0)
        nc.vector.memset(rhs0d8[64:65, :], 1.0)

        def emit_mms(grz, gn, gh, gy, rhs0, rhs0_8, wtn, wtrz, enc, want_y):
            """Gate pre-activation matmuls. rz path fp8, n/y path bf16."""
            hk = lambda k: hbf[:, (k - 1) * 64:k * 64]
            hk8 = lambda k: hbf8[:, (k - 1) * 64:k * 64]
            k0n = wtn[0][0:65, :]
            k0rz = wtrz[0][0:65, :]
            # phase A: all k0 matmuls (bias row; x rows for encoder)
            for m in range(8):  # r, z (fp8)
                nc.tensor.matmul(grz[:, 64 * m:64 * m + 64],
                                 k0rz[:, 128 * m:128 * m + 128],
                                 rhs0_8, start=(m == 0), stop=False)
            for m in range(4):  # i_n
                nc.tensor.matmul(gn[:, 64 * m:64 * m + 64],
                                 k0n[:, _N0 + 128 * m:_N0 + 128 * m + 128],
                                 rhs0, start=(m == 0),
                                 stop=(enc and m == 3))
            for m in range(4):  # h_n (bias only in k0)
                nc.tensor.matmul(gh[:, 64 * m:64 * m + 64],
                                 k0n[:, _H0 + 128 * m:_H0 + 128 * m + 128],
                                 rhs0, start=(m == 0), stop=False)
            if want_y:
                nc.tensor.matmul(gy[:, :], k0n[:, _Y0:_Y0 + 64],
                                 rhs0, start=True, stop=False)
            # phase B: r,z (fp8) — k-major so next step's k1/k2 can start on
            # hbf8 half-0 while half-1 gates still run
            for k in range(1, 5):
                for m in range(8):
                    nc.tensor.matmul(grz[:, 64 * m:64 * m + 64],
                                     wtrz[k][:, 128 * m:128 * m + 128],
                                     hk8(k), start=False,
                                     stop=(m == 7 and k == 4))
            # phase C_h: h_n (chain head input) then C_n: i_n (decoder only)
            for k in range(1, 5):
                for m in range(4):
                    nc.tensor.matmul(gh[:, 64 * m:64 * m + 64],
                                     wtn[k][:, _H0 + 128 * m:_H0 + 128 * m + 128],
                                     hk(k), start=False,
                                     stop=(m == 3 and k == 4))
            if not enc:
                for k in range(1, 5):
                    for m in range(4):
                        nc.tensor.matmul(gn[:, 64 * m:64 * m + 64],
                                         wtn[k][:, _N0 + 128 * m:_N0 + 128 * m + 128],
                                         hk(k), start=False,
                                         stop=(m == 3 and k == 4))
            # phase D: y over h k-tiles
            if want_y:
                for k in range(1, 5):
                    nc.tensor.matmul(gy[:, :], wtn[k][:, _Y0:_Y0 + 64],
                                     hk(k), start=False, stop=(k == 4))

        def emit_gates(grz, gn, gh, gy, ytile=None, yslot=0):
            """h' = h + (1-z)*(tanh(i_n + (1+tr)*gh) - h); tanh-only ACT.
            (1+tr)*gh = r*a_hn with gh pre-scaled 0.5; (1-z) = .5 - .5*tz."""
            trz = gp.tile([128, 512], bf16, tag="trz")
            nz2 = gp.tile([128, 256], bf16, tag="nz2")
            u2 = gp.tile([128, 256], f32, tag="u2")
            an = gp.tile([128, 256], f32, tag="an")
            tn = gp.tile([128, 256], bf16, tag="tn")
            dd = gp.tile([128, 256], bf16, tag="dd")
            ee = gp.tile([128, 256], bf16, tag="ee")
            tr_t = trz[:, 0:256]
            tz_t = trz[:, 256:512]
            H0, H1 = slice(0, 128), slice(128, 256)

            nc.scalar.activation(trz[:], grz[:, :], TANH)
            nc.vector.tensor_scalar(nz2[:], tz_t, -0.5, 0.5, MUL, ADD)
            for hh in (H0, H1):
                nc.vector.scalar_tensor_tensor(
                    u2[:, hh], tr_t[:, hh], 1.0, gh[:, hh], ADD, MUL)
                nc.vector.tensor_tensor(an[:, hh], u2[:, hh], gn[:, hh], ADD)
            nc.scalar.activation(tn[:, H0], an[:, H0], TANH)
            nc.scalar.activation(tn[:, H1], an[:, H1], TANH)
            for hh in (H0, H1):
                nc.vector.tensor_tensor(dd[:, hh], tn[:, hh], hbf[:, hh], SUB)
                nc.vector.tensor_tensor(ee[:, hh], nz2[:, hh], dd[:, hh], MUL)
                nc.vector.tensor_tensor(hbf[:, hh], hbf[:, hh], ee[:, hh], ADD)
                nc.vector.tensor_copy(hbf8[:, hh], hbf[:, hh])
            if ytile is not None:
                nc.vector.tensor_copy(
                    ytile[:, yslot * BC:(yslot + 1) * BC], gy[:, :])

        def alloc_psum():
            grz = psum.tile([128, 512], f32, tag="grz")
            gn = psum.tile([128, 256], f32, tag="gn")
            gh = psum.tile([128, 256], f32, tag="gh")
            gy = psum.tile([64, 64], f32, tag="gy")
            return grz, gn, gh, gy

        def enc_step(rhs0, rhs0_8):
            grz, gn, gh, gy = alloc_psum()
            emit_mms(grz, gn, gh, gy, rhs0, rhs0_8, wne, wrze,
                     enc=True, want_y=False)
            emit_gates(grz, gn, gh, gy)

        def dec_step(ytile, yslot):
            grz, gn, gh, gy = alloc_psum()
            emit_mms(grz, gn, gh, gy, rhs0d[0:65, :], rhs0d8[0:65, :],
                     wnd, wrzd, enc=False, want_y=True)
            emit_gates(grz, gn, gh, gy, ytile=ytile, yslot=yslot)

        PE = mybir.EngineType.PE
        DVE = mybir.EngineType.DVE

        # ---- encoder ----
        if use_loops:
            with tc.For_i(0, s_steps * BC, ue * BC, hint_engines=(PE, DVE)) as iv:
                xch = xpool.tile([65, ue * BC], bf16, tag="xch")
                nc.sync.dma_start(xch[:], x_ext[:, bass.ds(iv, ue * BC)])
                xch8 = xpool.tile([65, ue * BC], fp8, tag="xch8")
                nc.sync.dma_start(xch8[:], x8_ext[:, bass.ds(iv, ue * BC)])
                for j in range(ue):
                    enc_step(xch[:, j * BC:(j + 1) * BC],
                             xch8[:, j * BC:(j + 1) * BC])
        else:
            for i0 in range(0, s_steps, ue):
                xch = xpool.tile([65, ue * BC], bf16, tag="xch")
                nc.sync.dma_start(xch[:], x_ext[:, i0 * BC:(i0 + ue) * BC])
                xch8 = xpool.tile([65, ue * BC], fp8, tag="xch8")
                nc.sync.dma_start(xch8[:], x8_ext[:, i0 * BC:(i0 + ue) * BC])
                for j in range(ue):
                    enc_step(xch[:, j * BC:(j + 1) * BC],
                             xch8[:, j * BC:(j + 1) * BC])

        # ---- decoder (no bridge needed: Wl folded, no y feedback) ----
        if use_loops:
            with tc.For_i(0, t_steps * BC, ud * BC, hint_engines=(PE, DVE)) as iv:
                yt = ypool.tile([64, ud * BC], f32, tag="yt")
                for j in range(ud):
                    dec_step(yt, j)
                nc.sync.dma_start(y_ext[:, bass.ds(iv, ud * BC)], yt[:])
        else:
            for d0 in range(0, t_steps, ud):
                yt = ypool.tile([64, ud * BC], f32, tag="yt")
                for j in range(ud):
                    dec_step(yt, j)
                nc.sync.dma_start(y_ext[:, d0 * BC:(d0 + ud) * BC], yt[:])

        # ---- tail: y for the final hidden state -> slot T ----
        gy_t = psum.tile([64, 64], f32, tag="gy")
        nc.tensor.matmul(gy_t[:, :], wnd[0][0:65, _Y0:_Y0 + 64],
                         rhs0d[0:65, :], start=True, stop=False)
        for k in range(1, 5):
            nc.tensor.matmul(gy_t[:, :], wnd[k][:, _Y0:_Y0 + 64],
                             hbf[:, (k - 1) * 64:k * 64], start=False,
                             stop=(k == 4))
        ylast = ypool.tile([64, BC], f32, tag="ylast")
        nc.vector.tensor_copy(ylast[:], gy_t[:, :])
        nc.sync.dma_start(y_ext[:, t_steps * BC:(t_steps + 1) * BC], ylast[:])

    nc.compile()
    return nc


def run(nc, w_n, w_rz, x_cores, x8_cores, trace=False):
    """Execute on 8 cores; returns per-core y arrays and BassKernelResults."""
    from concourse.bass_utils import run_bass_kernel_spmd
    in_maps = [{"w_n": w_n, "w_rz": w_rz, "xh": x_cores[c], "xh8": x8_cores[c]}
               for c in range(NCORES)]
    res = run_bass_kernel_spmd(nc, in_maps, core_ids=list(range(NCORES)),
                               trace=trace)
    return [res.results[c]["y"] for c in range(NCORES)], res


_NC_CACHE = {}


def kernel(x, Wi, Wh, bi, bh, Wl, bl, targets=None, target_seq_len=T,
           teacher_forcing_rate=0, **_unused):
    x = np.asarray(x, np.float32)
    assert x.shape == (B, S, I), x.shape
    assert int(target_seq_len) == T
    w_n, w_rz = _build_weights(
        np.asarray(Wi, np.float32), np.asarray(Wh, np.float32),
        np.asarray(bi, np.float32), np.asarray(bh, np.float32),
        np.asarray(Wl, np.float32), np.asarray(bl, np.float32))
    xb, x8 = zip(*[_build_x(x[c * BC:(c + 1) * BC]) for c in range(NCORES)])

    key = (S, T)
    if key not in _NC_CACHE:
        _NC_CACHE[key] = build_program(S, T)
    ys, _ = run(_NC_CACHE[key], w_n, w_rz, list(xb), list(x8))

    out = np.empty((B, T, I), np.float32)
    for c in range(NCORES):
        yc = ys[c].reshape(64, T + 1, BC)[:, 1:, :]   # [I, T, BC]
        out[c * BC:(c + 1) * BC] = yc.transpose(2, 1, 0)
    return out


if __name__ == "__main__":
    import reference
    inputs = reference.setup_inputs()
    out = kernel(**{k: np.asarray(v) if hasattr(v, "shape") else v
                    for k, v in inputs.items()})
    print("kernel out", out.shape, out.dtype)


# revision 3
# speedup vs baseline: 1.1425x; 1.1425x over previous
"""Trainium2 Bass kernel for the GRU seq2seq AR model.

Model (reference): GRU encoder over S=1024 steps, then T=256 autoregressive
decoder steps (teacher_forcing_rate=0, so decoder input is always its own
previous output y = Wl @ h + bl).

Strategy:
  - Pure data parallel: batch 512 sharded 8 ways (64 rows/core), weights
    replicated, zero collectives.
  - Per step, ONE fused matmul group computes all gate pre-activations:
      g = lhsT.T @ [inp; 1; h]   (K = 64+1+512 -> 5 K-tiles of <=128)
    with M-columns laid out as [r|z|i_n|h_n|y] (512,512,512,512,64).
    A ones-row folds all biases into the matmul. r/z/h_n weight columns are
    pre-scaled by 0.5 so gates reduce to tanh only
    (sigmoid(a) = .5 + .5*tanh(.5a)), avoiding ACT table switches.
  - Decoder: Wl is FOLDED into the gate weights (inp = Wl@h + bl always), so
    the decoder has no serial y-feedback; the y chunk is output-only.
  - Gate math: h' = h + (1-z)*(tanh(i_n + (1+tr)*gh) - h) — 7 DVE ops,
    out-of-place ping-pong h tiles (in-place DVE add loses 2x mode).
  - bf16 matmul inputs, fp32 PSUM + fp32 gate math + bf16 hidden master.

Layouts (per core, BC = 64):
  w    DRAM [10, 128, 2112] bf16: k-tiles 0:5 encoder, 5:10 decoder.
  xh   DRAM [65, S*BC] bf16: rows 0:64 = x[t].T steps, row 64 = ones.
  y    DRAM [64, (T+1)*BC] f32: slot d holds Wl@h^{(d)}+bl ([I, BC] each).
"""

import numpy as np
import ml_dtypes

B, S, I, H, T = 512, 1024, 64, 512, 256
NCORES = 8
BC = B // NCORES
BF16 = ml_dtypes.bfloat16

# M-column blocks inside each 2112-col weight tile
_R0, _Z0, _N0, _H0, _Y0 = 0, 512, 1024, 1536, 2048
WCOLS = 2112


def _build_weights(Wi, Wh, bi, bh, Wl, bl):
    """lhsT tiles [10, 128, 2112] fp32 -> bf16.
    K rows: k0 = [x(64); ones(1)], k1..k4 = h chunks of 128.
    Tiles 0:5 = encoder, 5:10 = decoder (Wl folded)."""
    w = np.zeros((10, 128, WCOLS), np.float32)

    def fill(base, Wx, bx_r, bx_z, bx_n, Whh, Win_, x_has_w):
        # k0: x rows (encoder only) + bias row
        if x_has_w:
            w[base, 0:64, _R0:_R0 + 512] = 0.5 * Wx.T[:, 0:512]
            w[base, 0:64, _Z0:_Z0 + 512] = 0.5 * Wx.T[:, 512:1024]
            w[base, 0:64, _N0:_N0 + 512] = Wx.T[:, 1024:1536]
        w[base, 64, _R0:_R0 + 512] = 0.5 * bx_r
        w[base, 64, _Z0:_Z0 + 512] = 0.5 * bx_z
        w[base, 64, _N0:_N0 + 512] = bx_n
        w[base, 64, _H0:_H0 + 512] = 0.5 * bh[1024:1536]
        w[base, 64, _Y0:_Y0 + 64] = bl
        for c in range(4):
            hs = slice(128 * c, 128 * (c + 1))
            w[base + 1 + c, :, _R0:_R0 + 512] = 0.5 * Whh.T[hs, 0:512]
            w[base + 1 + c, :, _Z0:_Z0 + 512] = 0.5 * Whh.T[hs, 512:1024]
            if Win_ is not None:
                w[base + 1 + c, :, _N0:_N0 + 512] = Win_.T[hs, :]
            w[base + 1 + c, :, _H0:_H0 + 512] = 0.5 * Wh[1024:1536].T[hs, :]
            w[base + 1 + c, :, _Y0:_Y0 + 64] = Wl.T[hs, :]

    # encoder: gi from x via k0; gh from h
    fill(0, Wi, (bi + bh)[0:512], (bi + bh)[512:1024], bi[1024:1536],
         Wh[0:1024], None, x_has_w=True)
    # decoder: inp = Wl@h + bl folded -> all through h rows
    Wc = Wh[0:1024] + Wi[0:1024] @ Wl
    Win = Wi[1024:1536] @ Wl
    fill(5, Wi,
         (bi + bh)[0:512] + Wi[0:512] @ bl,
         (bi + bh)[512:1024] + Wi[512:1024] @ bl,
         bi[1024:1536] + Wi[1024:1536] @ bl,
         Wc, Win, x_has_w=False)
    return w.astype(BF16)


def _build_x(xc):
    """xc [BC, S, I] -> [65, S*BC] bf16 with ones row."""
    s = xc.shape[1]
    xt = np.ones((65, s, BC), np.float32)
    xt[0:64] = xc.transpose(2, 1, 0)
    return np.ascontiguousarray(xt.reshape(65, s * BC)).astype(BF16)


def build_program(s_steps=S, t_steps=T, ue=256, ud=64, use_loops=True):
    """Build the Bass/Tile program (shared by all 8 cores)."""
    from contextlib import ExitStack
    import concourse.bass as bass
    import concourse.bacc as bacc
    import concourse.mybir as mybir
    import concourse.tile as tile

    f32 = mybir.dt.float32
    bf16 = mybir.dt.bfloat16
    TANH = mybir.ActivationFunctionType.Tanh
    MUL = mybir.AluOpType.mult
    ADD = mybir.AluOpType.add
    SUB = mybir.AluOpType.subtract

    assert s_steps % (2 * ue) == 0 and t_steps % (2 * ud) == 0

    nc = bacc.Bacc("TRN2", target_bir_lowering=False, debug=False,
                   num_devices=NCORES)
    w_ext = nc.declare_dram_parameter("w", [10, 128, WCOLS], bf16, isOutput=False)
    x_ext = nc.declare_dram_parameter("xh", [65, s_steps * BC], bf16, isOutput=False)
    y_ext = nc.declare_dram_parameter("y", [64, (t_steps + 1) * BC], f32, isOutput=True)

    with ExitStack() as ctx:
        tc = ctx.enter_context(tile.TileContext(nc))
        state = ctx.enter_context(tc.tile_pool(name="state", bufs=1))
        wpool = ctx.enter_context(tc.tile_pool(name="wpool", bufs=1))
        xpool = ctx.enter_context(tc.tile_pool(name="xpool", bufs=2))
        ypool = ctx.enter_context(tc.tile_pool(name="ypool", bufs=2))
        gp = ctx.enter_context(tc.tile_pool(name="gates", bufs=3))
        psum = ctx.enter_context(tc.tile_pool(name="psum", bufs=2, space="PSUM"))

        wte, wtd = [], []
        for k in range(10):
            t_ = wpool.tile([128, WCOLS], bf16, tag=f"w{k}")
            nc.sync.dma_start(t_[:], w_ext[k, :, :])
            (wte if k < 5 else wtd).append(t_)

        # ping-pong h.T tiles (out-of-place update)
        hA = state.tile([128, 256], bf16, tag="hA")
        hB = state.tile([128, 256], bf16, tag="hB")
        rhs0d = state.tile([65, BC], bf16, tag="rhs0d")  # decoder k0 = [0...; 1]
        nc.vector.memset(hA[:], 0.0)
        nc.vector.memset(rhs0d[:], 0.0)
        nc.vector.memset(rhs0d[64:65, :], 1.0)

        def emit_mms(h_src, grz, gn, gh, gy, rhs0, wt, enc, want_y):
            """Per-bank psum tiles: grz [128,512] (r|z), gn [128,256] (i_n),
            gh [128,256] (h_n), gy [64,64]. One start (first MM) and one stop
            (last MM) per psum tile per step — start clears the whole bank."""
            hk = lambda k: h_src[:, (k - 1) * 64:k * 64]
            k0 = wt[0][0:65, :]
            # phase A: all k0 matmuls (bias row; x rows for encoder)
            for m in range(8):  # r, z
                nc.tensor.matmul(grz[:, 64 * m:64 * m + 64],
                                 k0[:, 128 * m:128 * m + 128],
                                 rhs0, start=(m == 0), stop=False)
            for m in range(4):  # i_n
                nc.tensor.matmul(gn[:, 64 * m:64 * m + 64],
                                 k0[:, _N0 + 128 * m:_N0 + 128 * m + 128],
                                 rhs0, start=(m == 0),
                                 stop=(enc and m == 3))
            for m in range(4):  # h_n (bias only in k0)
                nc.tensor.matmul(gh[:, 64 * m:64 * m + 64],
                                 k0[:, _H0 + 128 * m:_H0 + 128 * m + 128],
                                 rhs0, start=(m == 0), stop=False)
            if want_y:
                nc.tensor.matmul(gy[:, :], k0[:, _Y0:_Y0 + 64],
                                 rhs0, start=True, stop=False)
            # phase B: r,z — k-major so next step's k1/k2 can start on h
            # half-0 while half-1 gates still run
            for k in range(1, 5):
                for m in range(8):
                    nc.tensor.matmul(grz[:, 64 * m:64 * m + 64],
                                     wt[k][:, 128 * m:128 * m + 128],
                                     hk(k), start=False,
                                     stop=(m == 7 and k == 4))
            # phase C_h: h_n (chain head input) then C_n: i_n (decoder only)
            for k in range(1, 5):
                for m in range(4):
                    nc.tensor.matmul(gh[:, 64 * m:64 * m + 64],
                                     wt[k][:, _H0 + 128 * m:_H0 + 128 * m + 128],
                                     hk(k), start=False,
                                     stop=(m == 3 and k == 4))
            if not enc:
                for k in range(1, 5):
                    for m in range(4):
                        nc.tensor.matmul(gn[:, 64 * m:64 * m + 64],
                                         wt[k][:, _N0 + 128 * m:_N0 + 128 * m + 128],
                                         hk(k), start=False,
                                         stop=(m == 3 and k == 4))
            # phase D: y over h k-tiles
            if want_y:
                for k in range(1, 5):
                    nc.tensor.matmul(gy[:, :], wt[k][:, _Y0:_Y0 + 64],
                                     hk(k), start=False, stop=(k == 4))

        def emit_gates(h_src, h_dst, grz, gn, gh, gy, ytile=None, yslot=0):
            """h' = h + (1-z)*(tanh(i_n + (1+tr)*gh) - h), halved to shorten
            the serial chain. (1-z) = .5 - .5*tz with the 0.5-scaled weights;
            r*a_hn = (1+tr)*gh."""
            trz = gp.tile([128, 512], bf16, tag="trz")
            nz2 = gp.tile([128, 256], bf16, tag="nz2")
            u2 = gp.tile([128, 256], f32, tag="u2")
            an = gp.tile([128, 256], f32, tag="an")
            tn = gp.tile([128, 256], bf16, tag="tn")
            dd = gp.tile([128, 256], bf16, tag="dd")
            ee = gp.tile([128, 256], bf16, tag="ee")
            tr_t = trz[:, 0:256]
            tz_t = trz[:, 256:512]
            H0, H1 = slice(0, 128), slice(128, 256)

            # ACT: tanh only (no table switches)
            nc.scalar.activation(trz[:], grz[:, :], TANH)
            nc.vector.tensor_scalar(nz2[:], tz_t, -0.5, 0.5, MUL, ADD)
            # chain, halved: u2 -> an -> (tn on ACT) -> d -> e -> h_dst
            for hh in (H0, H1):
                nc.vector.scalar_tensor_tensor(
                    u2[:, hh], tr_t[:, hh], 1.0, gh[:, hh], ADD, MUL)
                nc.vector.tensor_tensor(an[:, hh], u2[:, hh], gn[:, hh], ADD)
            nc.scalar.activation(tn[:, H0], an[:, H0], TANH)
            nc.scalar.activation(tn[:, H1], an[:, H1], TANH)
            for hh in (H0, H1):
                nc.vector.tensor_tensor(dd[:, hh], tn[:, hh], h_src[:, hh], SUB)
                nc.vector.tensor_tensor(ee[:, hh], nz2[:, hh], dd[:, hh], MUL)
                nc.vector.tensor_tensor(h_dst[:, hh], h_src[:, hh], ee[:, hh], ADD)
            if ytile is not None:
                nc.vector.tensor_copy(
                    ytile[:, yslot * BC:(yslot + 1) * BC], gy[:, :])

        def alloc_psum():
            grz = psum.tile([128, 512], f32, tag="grz")
            gn = psum.tile([128, 256], f32, tag="gn")
            gh = psum.tile([128, 256], f32, tag="gh")
            gy = psum.tile([64, 64], f32, tag="gy")
            return grz, gn, gh, gy

        def enc_step(h_src, h_dst, rhs0):
            grz, gn, gh, gy = alloc_psum()
            emit_mms(h_src, grz, gn, gh, gy, rhs0, wte, enc=True, want_y=False)
            emit_gates(h_src, h_dst, grz, gn, gh, gy)

        def dec_step(h_src, h_dst, ytile, yslot):
            grz, gn, gh, gy = alloc_psum()
            emit_mms(h_src, grz, gn, gh, gy, rhs0d[0:65, :], wtd,
                     enc=False, want_y=True)
            emit_gates(h_src, h_dst, grz, gn, gh, gy, ytile=ytile, yslot=yslot)

        PE = mybir.EngineType.PE
        DVE = mybir.EngineType.DVE

        # ---- encoder ---- (ue even => h ping-pong consistent across iters)
        if use_loops:
            with tc.For_i(0, s_steps * BC, ue * BC, hint_engines=(PE, DVE)) as iv:
                xch = xpool.tile([65, ue * BC], bf16, tag="xch")
                nc.sync.dma_start(xch[:], x_ext[:, bass.ds(iv, ue * BC)])
                for j in range(ue):
                    hs, hd = (hA, hB) if j % 2 == 0 else (hB, hA)
                    enc_step(hs, hd, xch[:, j * BC:(j + 1) * BC])
        else:
            for i0 in range(0, s_steps, ue):
                xch = xpool.tile([65, ue * BC], bf16, tag="xch")
                nc.sync.dma_start(xch[:], x_ext[:, i0 * BC:(i0 + ue) * BC])
                for j in range(ue):
                    hs, hd = (hA, hB) if j % 2 == 0 else (hB, hA)
                    enc_step(hs, hd, xch[:, j * BC:(j + 1) * BC])

        # ---- decoder (no bridge needed: Wl folded, no y feedback) ----
        if use_loops:
            with tc.For_i(0, t_steps * BC, ud * BC, hint_engines=(PE, DVE)) as iv:
                yt = ypool.tile([64, ud * BC], f32, tag="yt")
                for j in range(ud):
                    hs, hd = (hA, hB) if j % 2 == 0 else (hB, hA)
                    dec_step(hs, hd, yt, j)
                nc.sync.dma_start(y_ext[:, bass.ds(iv, ud * BC)], yt[:])
        else:
            for d0 in range(0, t_steps, ud):
                yt = ypool.tile([64, ud * BC], f32, tag="yt")
                for j in range(ud):
                    hs, hd = (hA, hB) if j % 2 == 0 else (hB, hA)
                    dec_step(hs, hd, yt, j)
                nc.sync.dma_start(y_ext[:, d0 * BC:(d0 + ud) * BC], yt[:])

        # ---- tail: y for the final hidden state -> slot T ----
        gy_t = psum.tile([64, 64], f32, tag="gy")
        nc.tensor.matmul(gy_t[:, :], wtd[0][0:65, _Y0:_Y0 + 64],
                         rhs0d[0:65, :], start=True, stop=False)
        for k in range(1, 5):
            nc.tensor.matmul(gy_t[:, :], wtd[k][:, _Y0:_Y0 + 64],
                             hA[:, (k - 1) * 64:k * 64], start=False, stop=(k == 4))
        ylast = ypool.tile([64, BC], f32, tag="ylast")
        nc.vector.tensor_copy(ylast[:], gy_t[:, :])
        nc.sync.dma_start(y_ext[:, t_steps * BC:(t_steps + 1) * BC], ylast[:])

    nc.compile()
    return nc


def run(nc, w_bf, x_cores, trace=False):
    """Execute on 8 cores; returns per-core y arrays and BassKernelResults."""
    from concourse.bass_utils import run_bass_kernel_spmd
    in_maps = [{"w": w_bf, "xh": x_cores[c]} for c in range(NCORES)]
    res = run_bass_kernel_spmd(nc, in_maps, core_ids=list(range(NCORES)),
                               trace=trace)
    return [res.results[c]["y"] for c in range(NCORES)], res


_NC_CACHE = {}


def kernel(x, Wi, Wh, bi, bh, Wl, bl, targets=None, target_seq_len=T,
           teacher_forcing_rate=0, **_unused):
    x = np.asarray(x, np.float32)
    assert x.shape == (B, S, I), x.shape
    assert int(target_seq_len) == T
    w_bf = _build_weights(np.asarray(Wi, np.float32), np.asarray(Wh, np.float32),
                          np.asarray(bi, np.float32), np.asarray(bh, np.float32),
                          np.asarray(Wl, np.float32), np.asarray(bl, np.float32))
    x_cores = [_build_x(x[c * BC:(c + 1) * BC]) for c in range(NCORES)]

    key = (S, T)
    if key not in _NC_CACHE:
        _NC_CACHE[key] = build_program(S, T)
    ys, _ = run(_NC_CACHE[key], w_bf, x_cores)

    out = np.empty((B, T, I), np.float32)
    for c in range(NCORES):
        yc = ys[c].reshape(64, T + 1, BC)[:, 1:, :]   # [I, T, BC]
        out[c * BC:(c + 1) * BC] = yc.transpose(2, 1, 0)
    return out


if __name__ == "__main__":
    import reference
    inputs = reference.setup_inputs()
    out = kernel(**{k: np.asarray(v) if hasattr(v, "shape") else v
                    for k, v in inputs.items()})
    print("kernel out", out.shape, out.dtype)


# revision 10
# speedup vs baseline: 1.1562x; 1.0120x over previous
"""Trainium2 Bass kernel for the GRU seq2seq AR model.

Model (reference): GRU encoder over S=1024 steps, then T=256 autoregressive
decoder steps (teacher_forcing_rate=0, so decoder input is always its own
previous output y = Wl @ h + bl).

Strategy:
  - Pure data parallel: batch 512 sharded 8 ways (64 rows/core), weights
    replicated, zero collectives.
  - Per step, ONE fused matmul group computes all gate pre-activations:
      g = lhsT.T @ [inp; 1; h]   (K = 64+1+512 -> 5 K-tiles of <=128)
    with M-columns laid out as [r|z|i_n|h_n|y] (512,512,512,512,64).
    A ones-row folds all biases into the matmul. r/z/h_n weight columns are
    pre-scaled by 0.5 so gates reduce to tanh only
    (sigmoid(a) = .5 + .5*tanh(.5a)), avoiding ACT table switches.
  - Decoder: Wl is FOLDED into the gate weights (inp = Wl@h + bl always), so
    the decoder has no serial y-feedback; the y chunk is output-only.
  - bf16 matmul inputs, fp32 PSUM + fp32 gate math + bf16 hidden master.
  - Input/output DMA double-buffered (xpool/ypool bufs=2) with ue=256/ud=64
    chunks so chunk-boundary DMA overlaps compute.

Layouts (per core, BC = 64):
  w    DRAM [10, 128, 2112] bf16: k-tiles 0:5 encoder, 5:10 decoder.
  xh   DRAM [65, S*BC] bf16: rows 0:64 = x[t].T steps, row 64 = ones.
  y    DRAM [64, (T+1)*BC] f32: slot d holds Wl@h^{(d)}+bl ([I, BC] each).
"""

import numpy as np
import ml_dtypes

B, S, I, H, T = 512, 1024, 64, 512, 256
NCORES = 8
BC = B // NCORES
BF16 = ml_dtypes.bfloat16

# M-column blocks inside each 2112-col weight tile
_R0, _Z0, _N0, _H0, _Y0 = 0, 512, 1024, 1536, 2048
WCOLS = 2112


def _build_weights(Wi, Wh, bi, bh, Wl, bl):
    """lhsT tiles [10, 128, 2112] fp32 -> bf16.
    K rows: k0 = [x(64); ones(1)], k1..k4 = h chunks of 128.
    Tiles 0:5 = encoder, 5:10 = decoder (Wl folded)."""
    w = np.zeros((10, 128, WCOLS), np.float32)

    def fill(base, Wx, bx_r, bx_z, bx_n, Whh, Win_, x_has_w):
        # k0: x rows (encoder only) + bias row
        if x_has_w:
            w[base, 0:64, _R0:_R0 + 512] = 0.5 * Wx.T[:, 0:512]
            w[base, 0:64, _Z0:_Z0 + 512] = 0.5 * Wx.T[:, 512:1024]
            w[base, 0:64, _N0:_N0 + 512] = Wx.T[:, 1024:1536]
        w[base, 64, _R0:_R0 + 512] = 0.5 * bx_r
        w[base, 64, _Z0:_Z0 + 512] = 0.5 * bx_z
        w[base, 64, _N0:_N0 + 512] = bx_n
        w[base, 64, _H0:_H0 + 512] = 0.5 * bh[1024:1536]
        w[base, 64, _Y0:_Y0 + 64] = bl
        for c in range(4):
            hs = slice(128 * c, 128 * (c + 1))
            w[base + 1 + c, :, _R0:_R0 + 512] = 0.5 * Whh.T[hs, 0:512]
            w[base + 1 + c, :, _Z0:_Z0 + 512] = 0.5 * Whh.T[hs, 512:1024]
            if Win_ is not None:
                w[base + 1 + c, :, _N0:_N0 + 512] = Win_.T[hs, :]
            w[base + 1 + c, :, _H0:_H0 + 512] = 0.5 * Wh[1024:1536].T[hs, :]
            w[base + 1 + c, :, _Y0:_Y0 + 64] = Wl.T[hs, :]

    # encoder: gi from x via k0; gh from h
    fill(0, Wi, (bi + bh)[0:512], (bi + bh)[512:1024], bi[1024:1536],
         Wh[0:1024], None, x_has_w=True)
    # decoder: inp = Wl@h + bl folded -> all through h rows
    Wc = Wh[0:1024] + Wi[0:1024] @ Wl
    Win = Wi[1024:1536] @ Wl
    fill(5, Wi,
         (bi + bh)[0:512] + Wi[0:512] @ bl,
         (bi + bh)[512:1024] + Wi[512:1024] @ bl,
         bi[1024:1536] + Wi[1024:1536] @ bl,
         Wc, Win, x_has_w=False)
    return w.astype(BF16)


def _build_x(xc):
    """xc [BC, S, I] -> [65, S*BC] bf16 with ones row."""
    s = xc.shape[1]
    xt = np.ones((65, s, BC), np.float32)
    xt[0:64] = xc.transpose(2, 1, 0)
    return np.ascontiguousarray(xt.reshape(65, s * BC)).astype(BF16)


def build_program(s_steps=S, t_steps=T, ue=256, ud=64, use_loops=True):
    """Build the Bass/Tile program (shared by all 8 cores)."""
    from contextlib import ExitStack
    import concourse.bass as bass
    import concourse.bacc as bacc
    import concourse.mybir as mybir
    import concourse.tile as tile

    f32 = mybir.dt.float32
    bf16 = mybir.dt.bfloat16
    TANH = mybir.ActivationFunctionType.Tanh
    MUL = mybir.AluOpType.mult
    ADD = mybir.AluOpType.add
    SUB = mybir.AluOpType.subtract

    assert s_steps % (2 * ue) == 0 and t_steps % (2 * ud) == 0

    nc = bacc.Bacc("TRN2", target_bir_lowering=False, debug=False,
                   num_devices=NCORES)
    w_ext = nc.declare_dram_parameter("w", [10, 128, WCOLS], bf16, isOutput=False)
    x_ext = nc.declare_dram_parameter("xh", [65, s_steps * BC], bf16, isOutput=False)
    y_ext = nc.declare_dram_parameter("y", [64, (t_steps + 1) * BC], f32, isOutput=True)

    with ExitStack() as ctx:
        tc = ctx.enter_context(tile.TileContext(nc))
        state = ctx.enter_context(tc.tile_pool(name="state", bufs=1))
        wpool = ctx.enter_context(tc.tile_pool(name="wpool", bufs=1))
        xpool = ctx.enter_context(tc.tile_pool(name="xpool", bufs=2))
        ypool = ctx.enter_context(tc.tile_pool(name="ypool", bufs=2))
        gp = ctx.enter_context(tc.tile_pool(name="gates", bufs=3))
        psum = ctx.enter_context(tc.tile_pool(name="psum", bufs=2, space="PSUM"))

        wte, wtd = [], []
        for k in range(10):
            t_ = wpool.tile([128, WCOLS], bf16, tag=f"w{k}")
            nc.sync.dma_start(t_[:], w_ext[k, :, :])
            (wte if k < 5 else wtd).append(t_)

        hbf = state.tile([128, 256], bf16, tag="hbf")    # h.T chunks (bf16)
        rhs0d = state.tile([65, BC], bf16, tag="rhs0d")  # decoder k0 = [0...; 1]
        nc.vector.memset(hbf[:], 0.0)
        nc.vector.memset(rhs0d[:], 0.0)
        nc.vector.memset(rhs0d[64:65, :], 1.0)

        def emit_mms(grz, gn, gh, gy, rhs0, wt, enc, want_y):
            """Per-bank psum tiles: grz [128,512] (r|z), gn [128,256] (i_n),
            gh [128,256] (h_n), gy [64,64]. One start (first MM) and one stop
            (last MM) per psum tile per step — start clears the whole bank."""
            hk = lambda k: hbf[:, (k - 1) * 64:k * 64]
            k0 = wt[0][0:65, :]
            # phase A: all k0 matmuls (bias row; x rows for encoder)
            for m in range(8):  # r, z
                nc.tensor.matmul(grz[:, 64 * m:64 * m + 64],
                                 k0[:, 128 * m:128 * m + 128],
                                 rhs0, start=(m == 0), stop=False)
            for m in range(4):  # i_n
                nc.tensor.matmul(gn[:, 64 * m:64 * m + 64],
                                 k0[:, _N0 + 128 * m:_N0 + 128 * m + 128],
                                 rhs0, start=(m == 0),
                                 stop=(enc and m == 3))
            for m in range(4):  # h_n (bias only in k0)
                nc.tensor.matmul(gh[:, 64 * m:64 * m + 64],
                                 k0[:, _H0 + 128 * m:_H0 + 128 * m + 128],
                                 rhs0, start=(m == 0), stop=False)
            if want_y:
                nc.tensor.matmul(gy[:, :], k0[:, _Y0:_Y0 + 64],
                                 rhs0, start=True, stop=False)
            # phase B: r,z — k-major so next step's k1/k2 can start on h
            # half-0 while half-1 gates still run
            for k in range(1, 5):
                for m in range(8):
                    nc.tensor.matmul(grz[:, 64 * m:64 * m + 64],
                                     wt[k][:, 128 * m:128 * m + 128],
                                     hk(k), start=False,
                                     stop=(m == 7 and k == 4))
            # phase C_h: h_n (chain head input) then C_n: i_n (decoder only)
            for k in range(1, 5):
                for m in range(4):
                    nc.tensor.matmul(gh[:, 64 * m:64 * m + 64],
                                     wt[k][:, _H0 + 128 * m:_H0 + 128 * m + 128],
                                     hk(k), start=False,
                                     stop=(m == 3 and k == 4))
            if not enc:
                for k in range(1, 5):
                    for m in range(4):
                        nc.tensor.matmul(gn[:, 64 * m:64 * m + 64],
                                         wt[k][:, _N0 + 128 * m:_N0 + 128 * m + 128],
                                         hk(k), start=False,
                                         stop=(m == 3 and k == 4))
            # phase D: y over h k-tiles
            if want_y:
                for k in range(1, 5):
                    nc.tensor.matmul(gy[:, :], wt[k][:, _Y0:_Y0 + 64],
                                     hk(k), start=False, stop=(k == 4))

        def emit_gates(grz, gn, gh, gy, ytile=None, yslot=0):
            """Gate math, split in column halves to shorten the serial chain:
            an = i_n + (1+tr)*gh; h' = (1-z)*n + z*h. bf16 for SBUF-side
            tensors (2x/4x DVE modes); u2/an f32 (PSUM sources are f32)."""
            trz = gp.tile([128, 512], bf16, tag="trz")
            zz2 = gp.tile([128, 256], bf16, tag="zz2")
            nz2 = gp.tile([128, 256], bf16, tag="nz2")
            q = gp.tile([128, 256], bf16, tag="q")
            u2 = gp.tile([128, 256], f32, tag="u2")
            an = gp.tile([128, 256], f32, tag="an")
            tn = gp.tile([128, 256], bf16, tag="tn")
            mm1 = gp.tile([128, 256], bf16, tag="mm1")
            tr_t = trz[:, 0:256]
            tz_t = trz[:, 256:512]
            H0, H1 = slice(0, 128), slice(128, 256)

            # ACT: tanh only (no table switches)
            nc.scalar.activation(trz[:], grz[:, :], TANH)
            # prep ops (DVE, before the chain; only need tz_t)
            nc.vector.tensor_scalar(zz2[:], tz_t, 0.5, 0.5, MUL, ADD)
            nc.vector.tensor_scalar(nz2[:], tz_t, -0.5, 0.5, MUL, ADD)
            nc.vector.tensor_tensor(q[:], zz2[:], hbf[:], MUL)    # z*h (bf16)
            # chain, halved: u2 -> an -> (tn on ACT) -> mm1 -> hbf
            for hh in (H0, H1):
                nc.vector.scalar_tensor_tensor(
                    u2[:, hh], tr_t[:, hh], 1.0, gh[:, hh], ADD, MUL)
                nc.vector.tensor_tensor(an[:, hh], u2[:, hh], gn[:, hh], ADD)
            nc.scalar.activation(tn[:, H0], an[:, H0], TANH)
            nc.scalar.activation(tn[:, H1], an[:, H1], TANH)
            for hh in (H0, H1):
                nc.vector.tensor_tensor(mm1[:, hh], nz2[:, hh], tn[:, hh], MUL)
                nc.vector.tensor_tensor(hbf[:, hh], mm1[:, hh], q[:, hh], ADD)
            if ytile is not None:
                nc.vector.tensor_copy(
                    ytile[:, yslot * BC:(yslot + 1) * BC], gy[:, :])

        def alloc_psum():
            grz = psum.tile([128, 512], f32, tag="grz")
            gn = psum.tile([128, 256], f32, tag="gn")
            gh = psum.tile([128, 256], f32, tag="gh")
            gy = psum.tile([64, 64], f32, tag="gy")
            return grz, gn, gh, gy

        def enc_step(rhs0):
            grz, gn, gh, gy = alloc_psum()
            emit_mms(grz, gn, gh, gy, rhs0, wte, enc=True, want_y=False)
            emit_gates(grz, gn, gh, gy)

        def dec_step(ytile, yslot):
            grz, gn, gh, gy = alloc_psum()
            emit_mms(grz, gn, gh, gy, rhs0d[0:65, :], wtd, enc=False, want_y=True)
            emit_gates(grz, gn, gh, gy, ytile=ytile, yslot=yslot)

        PE = mybir.EngineType.PE
        DVE = mybir.EngineType.DVE

        # ---- encoder ----
        if use_loops:
            with tc.For_i(0, s_steps * BC, ue * BC, hint_engines=(PE, DVE)) as iv:
                xch = xpool.tile([65, ue * BC], bf16, tag="xch")
                nc.sync.dma_start(xch[:], x_ext[:, bass.ds(iv, ue * BC)])
                for j in range(ue):
                    enc_step(xch[:, j * BC:(j + 1) * BC])
        else:
            for i0 in range(0, s_steps, ue):
                xch = xpool.tile([65, ue * BC], bf16, tag="xch")
                nc.sync.dma_start(xch[:], x_ext[:, i0 * BC:(i0 + ue) * BC])
                for j in range(ue):
                    enc_step(xch[:, j * BC:(j + 1) * BC])

        # ---- decoder (no bridge needed: Wl folded, no y feedback) ----
        if use_loops:
            with tc.For_i(0, t_steps * BC, ud * BC, hint_engines=(PE, DVE)) as iv:
                yt = ypool.tile([64, ud * BC], f32, tag="yt")
                for j in range(ud):
                    dec_step(yt, j)
                nc.sync.dma_start(y_ext[:, bass.ds(iv, ud * BC)], yt[:])
        else:
            for d0 in range(0, t_steps, ud):
                yt = ypool.tile([64, ud * BC], f32, tag="yt")
                for j in range(ud):
                    dec_step(yt, j)
                nc.sync.dma_start(y_ext[:, d0 * BC:(d0 + ud) * BC], yt[:])

        # ---- tail: y for the final hidden state -> slot T ----
        gy_t = psum.tile([64, 64], f32, tag="gy")
        nc.tensor.matmul(gy_t[:, :], wtd[0][0:65, _Y0:_Y0 + 64],
                         rhs0d[0:65, :], start=True, stop=False)
        for k in range(1, 5):
            nc.tensor.matmul(gy_t[:, :], wtd[k][:, _Y0:_Y0 + 64],
                             hbf[:, (k - 1) * 64:k * 64], start=False, stop=(k == 4))
        ylast = ypool.tile([64, BC], f32, tag="ylast")
        nc.vector.tensor_copy(ylast[:], gy_t[:, :])
        nc.sync.dma_start(y_ext[:, t_steps * BC:(t_steps + 1) * BC], ylast[:])

    nc.compile()
    return nc


def run(nc, w_bf, x_cores, trace=False):
    """Execute on 8 cores; returns per-core y arrays and BassKernelResults."""
    from concourse.bass_utils import run_bass_kernel_spmd
    in_maps = [{"w": w_bf, "xh": x_cores[c]} for c in range(NCORES)]
    res = run_bass_kernel_spmd(nc, in_maps, core_ids=list(range(NCORES)),
                               trace=trace)
    return [res.results[c]["y"] for c in range(NCORES)], res


_NC_CACHE = {}


def kernel(x, Wi, Wh, bi, bh, Wl, bl, targets=None, target_seq_len=T,
           teacher_forcing_rate=0, **_unused):
    x = np.asarray(x, np.float32)
    assert x.shape == (B, S, I), x.shape
    assert int(target_seq_len) == T
    w_bf = _build_weights(np.asarray(Wi, np.float32), np.asarray(Wh, np.float32),
                          np.asarray(bi, np.float32), np.asarray(bh, np.float32),
                          np.asarray(Wl, np.float32), np.asarray(bl, np.float32))
    x_cores = [_build_x(x[c * BC:(c + 1) * BC]) for c in range(NCORES)]

    key = (S, T)
    if key not in _NC_CACHE:
        _NC_CACHE[key] = build_program(S, T)
    ys, _ = run(_NC_CACHE[key], w_bf, x_cores)

    out = np.empty((B, T, I), np.float32)
    for c in range(NCORES):
        yc = ys[c].reshape(64, T + 1, BC)[:, 1:, :]   # [I, T, BC]
        out[c * BC:(c + 1) * BC] = yc.transpose(2, 1, 0)
    return out


if __name__ == "__main__":
    import reference
    inputs = reference.setup_inputs()
    out = kernel(**{k: np.asarray(v) if hasattr(v, "shape") else v
                    for k, v in inputs.items()})
    print("kernel out", out.shape, out.dtype)
